# revision 1
# baseline (speedup 1.0000x reference)
"""Trainium2 Bass kernel for CrossAttention (RoPE, 16 heads, D=1024).

Sharding: data-parallel over (batch, query-half): core c handles batch c//2,
query rows [1024*(c%2), 1024*(c%2+1)).  Each core computes full k/v for its
batch (duplicated within the pair) so there is no cross-core communication;
the host gather is a pure concatenation.

Per-core dataflow (big tensors feature-major, i.e. transposed on host):
  qT  (D, T)   = wqT.T @ xT   + bq, then RoPE (host-permuted even/odd rows)
  kT  (D, S)   = wkT.T @ ctxT + bk, then RoPE
  v   (S, D+)  = ctxT.T @ wvT + bv, s-major, ones column per head (65-stride)
  S^T (S, T)   = kT_h.T @ qT_h        (K=64, head pairs packed via row groups)
  E   = exp(S^T/8)                     (ScalarE, straight out of 2-bank PSUM)
  PV  (65, T)  = [v_h | 1].T @ E       (K=128; PSUM row 64 = softmax denom Z)
  A^T_h = PV[0:64] * (1/Z broadcast)   (GpSimd partition_broadcast + DVE)
  outT (D, T)  = woT.T @ A^T + bo
All matmuls run as float32r (~1.6e-4 rel err, 4x faster than float32).
"""

import sys
import numpy as np

sys.path.insert(0, "/opt/trn_rl_repo")

import concourse.bacc as bacc  # noqa: E402
import concourse.tile as tile  # noqa: E402
from concourse import mybir  # noqa: E402

F32 = mybir.dt.float32
F32R = mybir.dt.float32r
AF = mybir.ActivationFunctionType

NHEAD = 16
DH = 64
B = 4
TQ = 2048
TKV = 2048
D = 1024
T_CORE = TQ // 2  # query rows per core
N_CORES = 8


def emit(nc, tc, hd, T, S, NH, phases=('q', 'kv', 'attn', 'out')):
    """Emit the per-core kernel.  T query rows, S kv rows, NH heads."""
    Dm = NH * DH
    NJ = Dm // 128          # 128-row feature blocks
    NSB = S // 128          # kv s-blocks
    NTC = T // 512          # 512-wide t chunks
    NQUAD = NH // 4
    scale = 1.0 / float(np.sqrt(DH))

    def load_blocked(pool, dram, rows, cols, dt, tag, eng=None):
        # dram (rows, cols) -> sbuf (128, (rows//128)*cols), block-major
        t = pool.tile([128, (rows // 128) * cols], dt, tag=tag)
        (eng or nc.sync).dma_start(
            t[:].rearrange("p (a c) -> p a c", a=rows // 128),
            dram[:].bitcast(dt).rearrange("(a p) c -> p a c", p=128),
        )
        return t

    with tc.tile_pool(name="consts", bufs=1) as consts:
        bq_sb = consts.tile([128, NJ], F32, tag="bq")
        nc.sync.dma_start(bq_sb[:], hd["bq_t"][:])
        bk_sb = consts.tile([128, NJ], F32, tag="bk")
        nc.sync.dma_start(bk_sb[:], hd["bk_t"][:])
        bo_sb = consts.tile([128, NJ], F32, tag="bo")
        nc.sync.dma_start(bo_sb[:], hd["bo_t"][:])
        ones_sb = consts.tile([128, (S // 128) * 4], F32R, tag="ones")
        nc.sync.dma_start(ones_sb[:], hd["ones_d"][:].bitcast(F32R))
        crepk = consts.tile([128, S], F32, tag="crepk")
        nc.scalar.dma_start(crepk[:], hd["crepk"][:])
        ssink = consts.tile([128, S], F32, tag="ssink")
        nc.scalar.dma_start(ssink[:], hd["ssink"][:])

        with tc.tile_pool(name="persist", bufs=1) as persist:
            ctx_sb = load_blocked(persist, hd["ctxT"], Dm, S, F32R, "ctx",
                                  eng=nc.scalar)

            # ================= phase 1: q projection + rope ==============
            with (
                tc.tile_pool(name="ph1", bufs=1) as ph1,
                tc.tile_pool(name="ph1w", bufs=2) as ph1w,
                tc.tile_pool(name="ph1ps", bufs=4, space="PSUM") as ph1ps,
                tc.tile_pool(name="rope1", bufs=1) as rope1,
                tc.tile_pool(name="q1o", bufs=2) as q1o,
            ):
                x_sb = load_blocked(ph1, hd["xT"], Dm, T, F32R, "x")
                crepq = ph1.tile([128, T], F32, tag="crepq")
                nc.sync.dma_start(crepq[:], hd["crepq"][:])
                ssinq = ph1.tile([128, T], F32, tag="ssinq")
                nc.sync.dma_start(ssinq[:], hd["ssinq"][:])
                for j in range(NJ if 'q' in phases else 0):
                    wq_j = ph1w.tile([128, NJ * 128], F32R, tag="wqj")
                    nc.sync.dma_start(
                        wq_j[:].rearrange("p (a c) -> p a c", a=NJ),
                        hd["wq"][:, j * 128:(j + 1) * 128].bitcast(F32R)
                        .rearrange("(a p) c -> p a c", p=128),
                    )
                    raw = rope1.tile([128, T], F32, tag="qraw")
                    for c in range(NTC):
                        ps = ph1ps.tile([128, 512], F32, tag="qps")
                        for i in range(NJ):
                            nc.tensor.matmul(
                                ps[:],
                                wq_j[:, i * 128:(i + 1) * 128],
                                x_sb[:, i * T + c * 512:i * T + c * 512 + 512],
                                start=(i == 0), stop=(i == NJ - 1),
                            )
                        nc.vector.tensor_scalar_add(
                            raw[:, c * 512:c * 512 + 512], ps[:],
                            bq_sb[:, j:j + 1],
                        )
                    shf = rope1.tile([128, T], F32, tag="qshf")
                    for g in range(4):
                        src = (g ^ 1) * 32
                        nc.sync.dma_start(shf[g * 32:g * 32 + 32, :],
                                          raw[src:src + 32, :])
                    nc.vector.tensor_mul(raw[:], raw[:], crepq[:])
                    nc.vector.tensor_mul(shf[:], shf[:], ssinq[:])
                    qstage = q1o.tile([128, T], F32R, tag="qstage")
                    nc.vector.tensor_add(qstage[:], raw[:], shf[:])
                    nc.sync.dma_start(
                        hd["q_t"][j * 128:(j + 1) * 128, :],
                        qstage[:].bitcast(F32))

            # ====== phases 2+3: per-quad kv projection + attention ======
            with (
                tc.tile_pool(name="kvw", bufs=1) as kvw,
                tc.tile_pool(name="kv", bufs=2) as kv,
                tc.tile_pool(name="ropek", bufs=1) as ropek,
                tc.tile_pool(name="esb", bufs=2) as esb,
                tc.tile_pool(name="zsb", bufs=1) as zsb,
                tc.tile_pool(name="stg", bufs=1) as stg,
                tc.tile_pool(name="qst", bufs=2) as qst,
                tc.tile_pool(name="psS", bufs=2, space="PSUM") as psS,
                tc.tile_pool(name="psV", bufs=4, space="PSUM") as psV,
            ):
                for qd in range(NQUAD if 'kv' in phases else 0):
                    # -- k projection for the quad's 256 feature rows --
                    wk_sb = kvw.tile([128, NJ * 256], F32R, tag="wk")
                    nc.sync.dma_start(
                        wk_sb[:].rearrange("p (a c) -> p a c", a=NJ),
                        hd["wk"][:, qd * 256:(qd + 1) * 256].bitcast(F32R)
                        .rearrange("(a p) c -> p a c", p=128),
                    )
                    wv_sb = kvw.tile([128, NJ * 256], F32R, tag="wv")
                    nc.sync.dma_start(
                        wv_sb[:].rearrange("p (a c) -> p a c", a=NJ),
                        hd["wv"][:, qd * 256:(qd + 1) * 256].bitcast(F32R)
                        .rearrange("(a p) c -> p a c", p=128),
                    )
                    bv_bc = kv.tile([128, 256], F32, tag="bvbc")
                    nc.sync.dma_start(
                        bv_bc[:], hd["bv_bcast"][:, qd * 256:(qd + 1) * 256])

                    kT = kv.tile([128, 2 * S], F32R, tag="kT")
                    for jj in range(2):
                        jglob = qd * 2 + jj
                        for c in range(S // 512):
                            kraw = ropek.tile([128, 512], F32, tag="kraw")
                            ps = psS.tile([128, 512], F32, tag="sps")
                            for i in range(NJ):
                                nc.tensor.matmul(
                                    ps[:],
                                    wk_sb[:, i * 256 + jj * 128:
                                          i * 256 + jj * 128 + 128],
                                    ctx_sb[:, i * S + c * 512:
                                           i * S + c * 512 + 512],
                                    start=(i == 0), stop=(i == NJ - 1),
                                )
                            nc.vector.tensor_scalar_add(
                                kraw[:], ps[:], bk_sb[:, jglob:jglob + 1])
                            kshf = ropek.tile([128, 512], F32, tag="kshf")
                            for g in range(4):
                                srcp = (g ^ 1) * 32
                                nc.sync.dma_start(kshf[g * 32:g * 32 + 32, :],
                                                  kraw[srcp:srcp + 32, :])
                            nc.vector.tensor_mul(
                                kraw[:], kraw[:],
                                crepk[:, c * 512:c * 512 + 512])
                            nc.vector.tensor_mul(
                                kshf[:], kshf[:],
                                ssink[:, c * 512:c * 512 + 512])
                            nc.vector.tensor_add(
                                kT[:, jj * S + c * 512:jj * S + c * 512 + 512],
                                kraw[:], kshf[:])

                    # -- v projection, s-major, 65-stride + ones column --
                    vq = kv.tile([128, NSB * 260], F32R, tag="vq")
                    for sb in range(NSB):
                        ps = psS.tile([128, 256], F32, tag="sps")
                        for i in range(NJ):
                            nc.tensor.matmul(
                                ps[:],
                                ctx_sb[:, i * S + sb * 128:
                                       i * S + sb * 128 + 128],
                                wv_sb[:, i * 256:(i + 1) * 256],
                                start=(i == 0), stop=(i == NJ - 1),
                            )
                        dst = vq[:, sb * 260:sb * 260 + 260].rearrange(
                            "p (h c) -> p h c", c=65)[:, :, 0:64]
                        nc.vector.tensor_add(
                            dst, ps[:].rearrange("p (h c) -> p h c", c=64),
                            bv_bc[:].rearrange("p (h c) -> p h c", c=64),
                        )
                    nc.vector.tensor_copy(
                        vq[:].rearrange("p (s h c) -> p s h c", h=4, c=65)
                        [:, :, :, 64:65],
                        ones_sb[:].rearrange("p (s h) -> p s h", h=4)
                        [:, :, :, None],
                    )

                    # -- attention for the quad's two head pairs --
                    for pr in range(2 if 'attn' in phases else 0):
                        qpair = qst.tile([128, T], F32R, tag="qpair")
                        nc.sync.dma_start(
                            qpair[:],
                            hd["q_t"][(qd * 2 + pr) * 128:
                                      (qd * 2 + pr + 1) * 128, :]
                            .bitcast(F32R))
                        pv_ps = [
                            [psV.tile([65, 512], F32, tag="pv", name="pv")
                             for _ in range(NTC)]
                            for _ in range(2)
                        ]
                        for sb in range(NSB):
                            e_t = [None, None]
                            for par in range(2):
                                hg = qd * 4 + 2 * pr + par
                                rows = slice(par * 64, par * 64 + 64)
                                sps = psS.tile([128, T], F32, tag="sps")
                                for c in range(NTC):
                                    nc.tensor.matmul(
                                        sps[:, c * 512:c * 512 + 512],
                                        kT[rows, pr * S + sb * 128:
                                           pr * S + sb * 128 + 128],
                                        qpair[rows, c * 512:c * 512 + 512],
                                        start=True, stop=True,
                                    )
                                et = esb.tile([128, T], F32R, tag="e")
                                nc.scalar.activation(et[:], sps[:], AF.Exp,
                                                     scale=scale)
                                e_t[par] = et
                            for par in range(2):
                                vcol = (2 * pr + par) * 65
                                for c in range(NTC):
                                    nc.tensor.matmul(
                                        pv_ps[par][c][:],
                                        vq[:, sb * 260 + vcol:
                                           sb * 260 + vcol + 65],
                                        e_t[par][:, c * 512:c * 512 + 512],
                                        start=(sb == 0), stop=(sb == NSB - 1),
                                    )
                        # normalize + store A^T
                        for par in range(2):
                            hg = qd * 4 + 2 * pr + par
                            for c in range(NTC):
                                ps = pv_ps[par][c]
                                zinv = zsb.tile([1, 512], F32, tag="zinv")
                                nc.vector.reciprocal(zinv[:], ps[64:65, :])
                                bc = zsb.tile([64, 512], F32, tag="bc")
                                nc.gpsimd.partition_broadcast(bc[:], zinv[:])
                                st = stg.tile([64, 512], F32, tag="st")
                                nc.vector.tensor_mul(st[:], ps[0:64, :], bc[:])
                                nc.sync.dma_start(
                                    hd["a_t"][hg * 64:hg * 64 + 64,
                                              c * 512:c * 512 + 512],
                                    st[:],
                                )

        # ================= phase 4: output projection =================
        with (
            tc.tile_pool(name="ph4", bufs=1) as ph4,
            tc.tile_pool(name="ph4ps", bufs=4, space="PSUM") as ph4ps,
            tc.tile_pool(name="ostg", bufs=4) as ostg,
        ):
            a_sb = load_blocked(ph4, hd["a_t"], Dm, T, F32R, "a")
            wo_sb = load_blocked(ph4, hd["wo"], Dm, Dm, F32R, "wo",
                                 eng=nc.scalar)
            for e in range(NJ if 'out' in phases else 0):
                for c in range(NTC):
                    ps = ph4ps.tile([128, 512], F32, tag="ops")
                    for i in range(NJ):
                        nc.tensor.matmul(
                            ps[:],
                            wo_sb[:, i * Dm + e * 128:i * Dm + (e + 1) * 128],
                            a_sb[:, i * T + c * 512:i * T + c * 512 + 512],
                            start=(i == 0), stop=(i == NJ - 1),
                        )
                    ot = ostg.tile([128, 512], F32, tag="ot")
                    nc.vector.tensor_scalar_add(ot[:], ps[:], bo_sb[:, e:e + 1])
                    nc.sync.dma_start(
                        hd["out_t"][e * 128:(e + 1) * 128,
                                    c * 512:c * 512 + 512],
                        ot[:],
                    )


def build(T=T_CORE, S=TKV, NH=NHEAD, reps=1,
          phases=('q', 'kv', 'attn', 'out')):
    Dm = NH * DH
    nc = bacc.Bacc("TRN2", target_bir_lowering=False, debug=False)
    hd = {}
    for name, shape in [
        ("xT", [Dm, T]), ("ctxT", [Dm, S]),
        ("wq", [Dm, Dm]), ("wk", [Dm, Dm]), ("wv", [Dm, Dm]), ("wo", [Dm, Dm]),
        ("crepk", [128, S]), ("ssink", [128, S]),
        ("crepq", [128, T]), ("ssinq", [128, T]),
        ("bq_t", [128, Dm // 128]), ("bk_t", [128, Dm // 128]),
        ("bv_bcast", [128, Dm]), ("bo_t", [128, Dm // 128]),
        ("ones_d", [128, (S // 128) * 4]),
    ]:
        hd[name] = nc.dram_tensor(name, shape, F32, kind="ExternalInput")
    hd["out_t"] = nc.dram_tensor("out_t", [Dm, T], F32, kind="ExternalOutput")
    hd["a_t"] = nc.dram_tensor("a_t", [Dm, T], F32)
    hd["q_t"] = nc.dram_tensor("q_t", [Dm, T], F32)

    with tile.TileContext(nc) as tc:
        for _ in range(reps):
            emit(nc, tc, hd, T, S, NH, phases=phases)
    nc.compile()
    return nc


def host_prep(x, context, Wq, bq, Wkv, bkv, Wo, bo, cos_tab, sin_tab,
              T=T_CORE, S=TKV, NH=NHEAD, n_cores=N_CORES):
    """Build the per-core input maps (pure layout work, no math).

    Core layout: core c -> batch c // (n_cores // B'), t-half c % 2 for the
    full-size problem.  For reduced test configs, n_cores maps onto
    (batch, t-chunk) pairs with T rows each.
    """
    Dm = NH * DH
    perm = np.concatenate(
        [h * DH + np.concatenate([np.arange(0, DH, 2), np.arange(1, DH, 2)])
         for h in range(NH)])
    c = np.ascontiguousarray
    wq = c(Wq[perm, :].T)
    wk = c(Wkv[0:Dm][perm, :].T)
    wv = c(Wkv[Dm:2 * Dm].T)
    wo = c(Wo.T)
    bq_t = c(bq[perm].reshape(Dm // 128, 128).T)
    bk_t = c(bkv[0:Dm][perm].reshape(Dm // 128, 128).T)
    bv_bcast = c(np.tile(bkv[Dm:2 * Dm].reshape(1, Dm), (128, 1)))
    bo_t = c(bo.reshape(Dm // 128, 128).T)

    def mk_tables(lo, hi):
        ct = cos_tab[lo:hi].T.astype(np.float32)
        st = sin_tab[lo:hi].T.astype(np.float32)
        return (c(np.tile(ct, (4, 1))),
                c(np.concatenate([-st, st, -st, st], axis=0)))

    crepk, ssink = mk_tables(0, S)

    shared = dict(wq=wq, wk=wk, wv=wv, wo=wo, bq_t=bq_t, bk_t=bk_t,
                  bv_bcast=bv_bcast, bo_t=bo_t, crepk=crepk, ssink=ssink,
                  ones_d=np.ones((128, (S // 128) * 4), np.float32))
    in_maps = []
    halves = n_cores // x.shape[0]
    for core in range(n_cores):
        b_i, th = divmod(core, halves)
        crepq, ssinq = mk_tables(th * T, (th + 1) * T)
        m = dict(shared)
        m.update(
            xT=c(x[b_i, th * T:(th + 1) * T, :].T),
            ctxT=c(context[b_i].T),
            crepq=crepq, ssinq=ssinq,
        )
        in_maps.append(m)
    return in_maps


_NC_CACHE = {}


def get_nc():
    if "nc" not in _NC_CACHE:
        _NC_CACHE["nc"] = build()
    return _NC_CACHE["nc"]




def make_runner(nc, n_cores=N_CORES):
    """Build a reusable jitted SPMD executor (device-resident inputs)."""
    import jax
    from jax.experimental.shard_map import shard_map
    from jax.sharding import Mesh, NamedSharding, PartitionSpec
    from concourse import bass2jax, mybir as _mybir

    bass2jax.install_neuronx_cc_hook()
    part_name = (nc.partition_id_tensor.name
                 if nc.partition_id_tensor else None)
    in_names, out_names, out_avals = [], [], []
    for alloc in nc.m.functions[0].allocations:
        if not isinstance(alloc, _mybir.MemoryLocationSet):
            continue
        name = alloc.memorylocations[0].name
        if alloc.kind == "ExternalInput":
            if name == part_name:
                continue
            in_names.append(name)
        elif alloc.kind == "ExternalOutput":
            out_names.append(name)
            out_avals.append(jax.core.ShapedArray(
                tuple(alloc.tensor_shape), _mybir.dt.np(alloc.dtype)))
    n_params = len(in_names)
    all_in = in_names + out_names
    if part_name is not None:
        all_in = all_in + [part_name]

    def _body(*args):
        ops = list(args)
        if part_name is not None:
            ops.append(bass2jax.partition_id_tensor())
        outs = bass2jax._bass_exec_p.bind(
            *ops,
            out_avals=tuple(out_avals),
            in_names=tuple(all_in),
            out_names=tuple(out_names),
            lowering_input_output_aliases=(),
            sim_require_finite=True,
            sim_require_nnan=True,
            nc=nc,
        )
        return tuple(outs)

    devices = jax.devices()[:n_cores]
    mesh = Mesh(np.asarray(devices), ("core",))
    nouts = len(out_names)
    sharded = jax.jit(
        shard_map(_body, mesh=mesh,
                  in_specs=(PartitionSpec("core"),) * (n_params + nouts),
                  out_specs=(PartitionSpec("core"),) * nouts,
                  check_rep=False),
        keep_unused=True,
    )
    sh = NamedSharding(mesh, PartitionSpec("core"))

    def put(in_maps):
        args = [np.concatenate([m[name] for m in in_maps], axis=0)
                for name in in_names[:n_params]]
        for av in out_avals:
            args.append(np.zeros((n_cores * av.shape[0],) + av.shape[1:],
                                 av.dtype))
        return [jax.device_put(a, sh) for a in args]

    def run(args):
        outs = sharded(*args)
        jax.block_until_ready(outs)
        return outs

    def gather(outs):
        return [
            {name: np.asarray(outs[i]).reshape(n_cores, *out_avals[i].shape)[c]
             for i, name in enumerate(out_names)}
            for c in range(n_cores)
        ]

    return put, run, gather


def get_runner():
    if "runner" not in _NC_CACHE:
        _NC_CACHE["runner"] = make_runner(get_nc())
    return _NC_CACHE["runner"]


def kernel(x, context, Wq, bq, Wkv, bkv, Wo, bo, cos_tab, sin_tab):
    args = [np.asarray(a, dtype=np.float32) for a in
            (x, context, Wq, bq, Wkv, bkv, Wo, bo, cos_tab, sin_tab)]
    in_maps = host_prep(*args)
    put, run, gather = get_runner()
    res = gather(run(put(in_maps)))
    out = np.empty((B, TQ, D), dtype=np.float32)
    for core in range(N_CORES):
        b_i, th = divmod(core, 2)
        out[b_i, th * T_CORE:(th + 1) * T_CORE, :] = res[core]["out_t"].T
    return out



# revision 2
# speedup vs baseline: 103.6449x; 103.6449x over previous
"""Trainium2 Bass kernel for CrossAttention (RoPE, 16 heads, D=1024) — v2.

Sharding: data-parallel over (batch, query-half): core c handles batch c//2,
query rows [1024*(c%2), 1024*(c%2+1)).  No cross-core communication; the
host gather is a pure concatenation.

v2 changes vs baseline:
  - all-bf16 data path (host pre-converts inputs/weights): halves DMA and
    SBUF, 2x DVE throughput, same 1 col/cycle matmul stream rate
  - q and A^T stay SBUF-resident (no q_t/a_t DRAM round trips)
  - PV matmul in fp8e4 + DoubleRow: contracts 256 kv rows per matmul
    (2 s-blocks interleaved), halving PV matmul count and cycles
  - all weights loaded up front; per-pair tile granularity so the Tile
    scheduler overlaps k/v-projection of quad q+1 under the ACT-bound
    attention of pair p
  - PSUM: psS bufs=2 x [128,1024] (4 banks) + psV bufs=4 x [65,512]
    (4 banks) = exactly 8 banks
"""

import sys
import numpy as np

sys.path.insert(0, "/opt/trn_rl_repo")

import concourse.bacc as bacc  # noqa: E402
import concourse.tile as tile  # noqa: E402
from concourse import mybir  # noqa: E402

F32 = mybir.dt.float32
BF16 = mybir.dt.bfloat16
FP8 = mybir.dt.float8e4
AF = mybir.ActivationFunctionType
PM = mybir.MatmulPerfMode

NHEAD = 16
DH = 64
B = 4
TQ = 2048
TKV = 2048
D = 1024
T_CORE = TQ // 2
N_CORES = 8

NJ = 8          # 128-row feature blocks
T = T_CORE      # 1024 query rows per core
S = TKV         # 2048 kv rows
NSB = S // 128  # 16 kv s-blocks
NSBP = NSB // 2  # 8 s-block pairs (DoubleRow)
NPAIR = 8       # head pairs
SCALE = 1.0 / float(np.sqrt(DH))


def emit(nc, tc, hd, phases=('q', 'kv', 'attn', 'out')):
    dmaq = [nc.sync, nc.gpsimd]

    with (
        tc.tile_pool(name="consts", bufs=1) as consts,
        tc.tile_pool(name="wpool", bufs=1) as wpool,
        tc.tile_pool(name="xin", bufs=1) as xin,
        tc.tile_pool(name="qres", bufs=1) as qres,
        tc.tile_pool(name="ares", bufs=1) as ares,
        tc.tile_pool(name="ktres", bufs=1) as ktres,
        tc.tile_pool(name="vqres", bufs=1) as vqres,
    ):
        # ---- input / weight DMAs (sync: activations, scalar: weights) ----
        x_t = []
        for i in range(NJ):
            t = xin.tile([128, T], BF16, tag=f"x{i}")
            nc.sync.dma_start(t[:], hd["xT"][i * 128:(i + 1) * 128, :])
            x_t.append(t)
        crepq = consts.tile([128, T], BF16, tag="crepq")
        nc.sync.dma_start(crepq[:], hd["crepq"][:])
        ssinq = consts.tile([128, T], BF16, tag="ssinq")
        nc.sync.dma_start(ssinq[:], hd["ssinq"][:])
        bq_sb = consts.tile([128, NJ], F32, tag="bq")
        nc.sync.dma_start(bq_sb[:], hd["bq_t"][:])

        wq_t = []
        for j in range(NJ):
            t = wpool.tile([128, NJ * 128], BF16, tag=f"wq{j}")
            nc.scalar.dma_start(
                t[:].rearrange("p (a c) -> p a c", a=NJ),
                hd["wq"][:, j * 128:(j + 1) * 128]
                .rearrange("(a p) c -> p a c", p=128),
            )
            wq_t.append(t)

        ctx_t = []
        for i in range(NJ):
            t = xin.tile([128, S], BF16, tag=f"ctx{i}")
            nc.sync.dma_start(t[:], hd["ctxT"][i * 128:(i + 1) * 128, :])
            ctx_t.append(t)
        crepk = consts.tile([128, S], BF16, tag="crepk")
        nc.sync.dma_start(crepk[:], hd["crepk"][:])
        ssink = consts.tile([128, S], BF16, tag="ssink")
        nc.sync.dma_start(ssink[:], hd["ssink"][:])
        bk_sb = consts.tile([128, NJ], F32, tag="bk")
        nc.sync.dma_start(bk_sb[:], hd["bk_t"][:])

        wk_t = []
        for j in range(NJ):
            t = wpool.tile([128, NJ * 128], BF16, tag=f"wk{j}")
            nc.scalar.dma_start(
                t[:].rearrange("p (a c) -> p a c", a=NJ),
                hd["wk"][:, j * 128:(j + 1) * 128]
                .rearrange("(a p) c -> p a c", p=128),
            )
            wk_t.append(t)
        wv_t = []
        for i in range(NJ):
            t = wpool.tile([128, D], BF16, tag=f"wv{i}")
            nc.scalar.dma_start(t[:], hd["wv"][i * 128:(i + 1) * 128, :])
            wv_t.append(t)
        bv_sb = consts.tile([128, D], F32, tag="bv")
        nc.scalar.dma_start(bv_sb[:], hd["bv_bcast"][:])
        ones_sb = consts.tile([128, 64], BF16, tag="ones")
        nc.scalar.dma_start(ones_sb[:], hd["ones_c"][:])
        wo_t = wpool.tile([128, NJ * D], BF16, tag="wo")
        nc.scalar.dma_start(
            wo_t[:].rearrange("p (a c) -> p a c", a=NJ),
            hd["wo"][:].rearrange("(a p) c -> p a c", p=128),
        )
        bo_sb = consts.tile([128, NJ], F32, tag="bo")
        nc.scalar.dma_start(bo_sb[:], hd["bo_t"][:])

        q_t = [qres.tile([128, T], BF16, tag=f"q{j}", name=f"q{j}")
               for j in range(NJ)]
        a_t = [ares.tile([128, T], BF16, tag=f"a{j}", name=f"a{j}")
               for j in range(NJ)]
        kt_t = [ktres.tile([128, S], BF16, tag=f"kt{j}", name=f"kt{j}")
                for j in range(NJ)]
        vq_t = [vqres.tile([128, NSBP * 2 * 4 * 80], FP8, tag=f"vq{qd}",
                           name=f"vq{qd}")
                for qd in range(4)]

        with (
            tc.tile_pool(name="psS", bufs=2, space="PSUM") as psS,
            tc.tile_pool(name="psV", bufs=4, space="PSUM") as psV,
            tc.tile_pool(name="rp", bufs=2) as rp,
            tc.tile_pool(name="esb", bufs=3) as esb,
            tc.tile_pool(name="zsb", bufs=2) as zsb,
            tc.tile_pool(name="ostg", bufs=4) as ostg,
        ):
            # ================= phase 1: q projection + rope ==============
            for j in range(NJ if 'q' in phases else 0):
                ps = psS.tile([128, T], F32, tag="ps")
                for c in range(2):
                    for i in range(NJ):
                        nc.tensor.matmul(
                            ps[:, c * 512:c * 512 + 512],
                            wq_t[j][:, i * 128:(i + 1) * 128],
                            x_t[i][:, c * 512:c * 512 + 512],
                            start=(i == 0), stop=(i == NJ - 1),
                        )
                raw = rp.tile([128, T], BF16, tag="raw")
                nc.vector.tensor_scalar_add(raw[:], ps[:], bq_sb[:, j:j + 1])
                shf = rp.tile([128, T], BF16, tag="shf")
                for g in range(4):
                    src = (g ^ 1) * 32
                    dmaq[g % 2].dma_start(shf[g * 32:g * 32 + 32, :],
                                          raw[src:src + 32, :])
                nc.vector.tensor_mul(raw[:], raw[:], crepq[:])
                nc.vector.tensor_mul(shf[:], shf[:], ssinq[:])
                nc.vector.tensor_add(q_t[j][:], raw[:], shf[:])

            # ========= per-quad kv projection + per-pair attention ========
            for qd in range(4 if 'kv' in phases else 0):
                # -- k projection for the quad's two pair-blocks --
                for jj in range(2):
                    j = qd * 2 + jj
                    for c in range(S // 512):
                        ps = psS.tile([128, 512], F32, tag="ps")
                        for i in range(NJ):
                            nc.tensor.matmul(
                                ps[:],
                                wk_t[j][:, i * 128:(i + 1) * 128],
                                ctx_t[i][:, c * 512:c * 512 + 512],
                                start=(i == 0), stop=(i == NJ - 1),
                            )
                        kraw = rp.tile([128, 512], BF16, tag="raw")
                        nc.vector.tensor_scalar_add(kraw[:], ps[:],
                                                    bk_sb[:, j:j + 1])
                        kshf = rp.tile([128, 512], BF16, tag="shf")
                        for g in range(4):
                            src = (g ^ 1) * 32
                            dmaq[g % 2].dma_start(kshf[g * 32:g * 32 + 32, :],
                                                  kraw[src:src + 32, :])
                        nc.vector.tensor_mul(
                            kraw[:], kraw[:], crepk[:, c * 512:c * 512 + 512])
                        nc.vector.tensor_mul(
                            kshf[:], kshf[:], ssink[:, c * 512:c * 512 + 512])
                        nc.vector.tensor_add(
                            kt_t[j][:, c * 512:c * 512 + 512], kraw[:], kshf[:])

                # -- v projection: s-major fp8, DR-interleaved, ones col --
                vql = vq_t[qd][:].rearrange("p (s t h d) -> p s t h d",
                                            s=NSBP, t=2, h=4)
                for sb in range(NSB):
                    sbp, par = sb // 2, sb % 2
                    ps = psS.tile([128, 256], F32, tag="ps")
                    for i in range(NJ):
                        nc.tensor.matmul(
                            ps[:],
                            ctx_t[i][:, sb * 128:sb * 128 + 128],
                            wv_t[i][:, qd * 256:(qd + 1) * 256],
                            start=(i == 0), stop=(i == NJ - 1),
                        )
                    nc.vector.tensor_add(
                        vql[:, sbp, par, :, 0:64],
                        ps[:].rearrange("p (h d) -> p h d", d=64),
                        bv_sb[:, qd * 256:(qd + 1) * 256]
                        .rearrange("p (h d) -> p h d", d=64),
                    )
                nc.vector.tensor_copy(
                    vql[:, :, :, :, 64:65],
                    ones_sb[:].rearrange("p (s t h) -> p s t h", s=NSBP, t=2)
                    [:, :, :, :, None],
                )

                # -- attention for the quad's two head pairs --
                for jj in range(2 if 'attn' in phases else 0):
                    pair = qd * 2 + jj
                    pv_ps = [[psV.tile([65, 512], F32, tag="pv", name="pv")
                              for _ in range(2)] for _ in range(2)]
                    for sbp in range(NSBP):
                        e2 = [None, None]
                        for par in range(2):
                            e2[par] = esb.tile([128, 2 * T], FP8, tag="e2", name="e2")
                        for parity in range(2):
                            sb = sbp * 2 + parity
                            for par in range(2):
                                rows = slice(par * 64, par * 64 + 64)
                                sps = psS.tile([128, T], F32, tag="ps")
                                for c in range(2):
                                    nc.tensor.matmul(
                                        sps[:, c * 512:c * 512 + 512],
                                        kt_t[pair][rows,
                                                   sb * 128:sb * 128 + 128],
                                        q_t[pair][rows,
                                                  c * 512:c * 512 + 512],
                                        start=True, stop=True,
                                    )
                                nc.scalar.activation(
                                    e2[par][:, parity * T:parity * T + T],
                                    sps[:], AF.Exp, scale=SCALE)
                        for par in range(2):
                            hq = jj * 2 + par
                            lhs = vql[:, sbp, :, hq, 0:65]
                            for c in range(2):
                                nc.tensor.matmul(
                                    pv_ps[par][c][:],
                                    lhs,
                                    e2[par][:].rearrange(
                                        "p (t n) -> p t n", t=2)
                                    [:, :, c * 512:c * 512 + 512],
                                    start=(sbp == 0), stop=(sbp == NSBP - 1),
                                    perf_mode=PM.DoubleRow,
                                )
                    for par in range(2):
                        for c in range(2):
                            ps = pv_ps[par][c]
                            zinv = zsb.tile([1, 512], F32, tag="zinv")
                            nc.vector.reciprocal(zinv[:], ps[64:65, :])
                            bc = zsb.tile([64, 512], F32, tag="bc")
                            nc.gpsimd.partition_broadcast(bc[:], zinv[:])
                            nc.vector.tensor_mul(
                                a_t[pair][par * 64:par * 64 + 64,
                                          c * 512:c * 512 + 512],
                                ps[0:64, :], bc[:])

            # ================= phase 4: output projection =================
            for e in range(NJ if 'out' in phases else 0):
                for c in range(2):
                    ps = psS.tile([128, 512], F32, tag="ps")
                    for i in range(NJ):
                        nc.tensor.matmul(
                            ps[:],
                            wo_t[:, i * D + e * 128:i * D + e * 128 + 128],
                            a_t[i][:, c * 512:c * 512 + 512],
                            start=(i == 0), stop=(i == NJ - 1),
                        )
                    ot = ostg.tile([128, 512], BF16, tag="ot")
                    nc.vector.tensor_scalar_add(ot[:], ps[:], bo_sb[:, e:e + 1])
                    dmaq[(e * 2 + c) % 2].dma_start(
                        hd["out_t"][e * 128:(e + 1) * 128,
                                    c * 512:c * 512 + 512],
                        ot[:],
                    )


def build(reps=1, phases=('q', 'kv', 'attn', 'out')):
    nc = bacc.Bacc("TRN2", target_bir_lowering=False, debug=False)
    hd = {}
    for name, shape, dt in [
        ("xT", [D, T], BF16), ("ctxT", [D, S], BF16),
        ("wq", [D, D], BF16), ("wk", [D, D], BF16),
        ("wv", [D, D], BF16), ("wo", [D, D], BF16),
        ("crepq", [128, T], BF16), ("ssinq", [128, T], BF16),
        ("crepk", [128, S], BF16), ("ssink", [128, S], BF16),
        ("bq_t", [128, NJ], F32), ("bk_t", [128, NJ], F32),
        ("bv_bcast", [128, D], F32), ("bo_t", [128, NJ], F32),
        ("ones_c", [128, 64], BF16),
    ]:
        hd[name] = nc.dram_tensor(name, shape, dt, kind="ExternalInput")
    hd["out_t"] = nc.dram_tensor("out_t", [D, T], BF16, kind="ExternalOutput")

    with tile.TileContext(nc) as tc:
        for _ in range(reps):
            emit(nc, tc, hd, phases=phases)
    nc.compile()
    return nc


def host_prep(x, context, Wq, bq, Wkv, bkv, Wo, bo, cos_tab, sin_tab):
    """Build the per-core input maps (layout + dtype conversion only)."""
    import ml_dtypes
    BF = ml_dtypes.bfloat16
    Dm = D
    perm = np.concatenate(
        [h * DH + np.concatenate([np.arange(0, DH, 2), np.arange(1, DH, 2)])
         for h in range(NHEAD)])
    c = np.ascontiguousarray
    wq = c(Wq[perm, :].T.astype(BF))
    wk = c(Wkv[0:Dm][perm, :].T.astype(BF))
    wv = c(Wkv[Dm:2 * Dm].T.astype(BF))
    wo = c(Wo.T.astype(BF))
    bq_t = c(bq[perm].reshape(NJ, 128).T.astype(np.float32))
    bk_t = c(bkv[0:Dm][perm].reshape(NJ, 128).T.astype(np.float32))
    bv_bcast = c(np.tile(bkv[Dm:2 * Dm].reshape(1, Dm), (128, 1))
                 .astype(np.float32))
    bo_t = c(bo.reshape(NJ, 128).T.astype(np.float32))

    def mk_tables(lo, hi):
        ct = cos_tab[lo:hi].T.astype(np.float32)
        st = sin_tab[lo:hi].T.astype(np.float32)
        return (c(np.tile(ct, (4, 1)).astype(BF)),
                c(np.concatenate([-st, st, -st, st], axis=0).astype(BF)))

    crepk, ssink = mk_tables(0, S)
    shared = dict(wq=wq, wk=wk, wv=wv, wo=wo, bq_t=bq_t, bk_t=bk_t,
                  bv_bcast=bv_bcast, bo_t=bo_t, crepk=crepk, ssink=ssink,
                  ones_c=np.ones((128, 64), BF))
    in_maps = []
    for core in range(N_CORES):
        b_i, th = divmod(core, 2)
        crepq, ssinq = mk_tables(th * T, (th + 1) * T)
        m = dict(shared)
        m.update(
            xT=c(x[b_i, th * T:(th + 1) * T, :].T.astype(BF)),
            ctxT=c(context[b_i].T.astype(BF)),
            crepq=crepq, ssinq=ssinq,
        )
        in_maps.append(m)
    return in_maps


_NC_CACHE = {}


def get_nc():
    if "nc" not in _NC_CACHE:
        _NC_CACHE["nc"] = build()
    return _NC_CACHE["nc"]


def make_runner(nc, n_cores=N_CORES):
    """Build a reusable jitted SPMD executor (device-resident inputs)."""
    import jax
    from jax.experimental.shard_map import shard_map
    from jax.sharding import Mesh, NamedSharding, PartitionSpec
    from concourse import bass2jax, mybir as _mybir

    bass2jax.install_neuronx_cc_hook()
    part_name = (nc.partition_id_tensor.name
                 if nc.partition_id_tensor else None)
    in_names, out_names, out_avals = [], [], []
    for alloc in nc.m.functions[0].allocations:
        if not isinstance(alloc, _mybir.MemoryLocationSet):
            continue
        name = alloc.memorylocations[0].name
        if alloc.kind == "ExternalInput":
            if name == part_name:
                continue
            in_names.append(name)
        elif alloc.kind == "ExternalOutput":
            out_names.append(name)
            out_avals.append(jax.core.ShapedArray(
                tuple(alloc.tensor_shape), _mybir.dt.np(alloc.dtype)))
    n_params = len(in_names)
    all_in = in_names + out_names
    if part_name is not None:
        all_in = all_in + [part_name]

    def _body(*args):
        ops = list(args)
        if part_name is not None:
            ops.append(bass2jax.partition_id_tensor())
        outs = bass2jax._bass_exec_p.bind(
            *ops,
            out_avals=tuple(out_avals),
            in_names=tuple(all_in),
            out_names=tuple(out_names),
            lowering_input_output_aliases=(),
            sim_require_finite=True,
            sim_require_nnan=True,
            nc=nc,
        )
        return tuple(outs)

    devices = jax.devices()[:n_cores]
    mesh = Mesh(np.asarray(devices), ("core",))
    nouts = len(out_names)
    sharded = jax.jit(
        shard_map(_body, mesh=mesh,
                  in_specs=(PartitionSpec("core"),) * (n_params + nouts),
                  out_specs=(PartitionSpec("core"),) * nouts,
                  check_rep=False),
        keep_unused=True,
    )
    sh = NamedSharding(mesh, PartitionSpec("core"))

    def put(in_maps):
        args = [np.concatenate([m[name] for m in in_maps], axis=0)
                for name in in_names[:n_params]]
        for av in out_avals:
            args.append(np.zeros((n_cores * av.shape[0],) + av.shape[1:],
                                 av.dtype))
        return [jax.device_put(a, sh) for a in args]

    def run(args):
        outs = sharded(*args)
        jax.block_until_ready(outs)
        return outs

    def gather(outs):
        return [
            {name: np.asarray(outs[i]).reshape(n_cores, *out_avals[i].shape)[c]
             for i, name in enumerate(out_names)}
            for c in range(n_cores)
        ]

    return put, run, gather


def get_runner():
    if "runner" not in _NC_CACHE:
        _NC_CACHE["runner"] = make_runner(get_nc())
    return _NC_CACHE["runner"]


def kernel(x, context, Wq, bq, Wkv, bkv, Wo, bo, cos_tab, sin_tab):
    args = [np.asarray(a, dtype=np.float32) for a in
            (x, context, Wq, bq, Wkv, bkv, Wo, bo, cos_tab, sin_tab)]
    in_maps = host_prep(*args)
    put, run, gather = get_runner()
    res = gather(run(put(in_maps)))
    out = np.empty((B, TQ, D), dtype=np.float32)
    for core in range(N_CORES):
        b_i, th = divmod(core, 2)
        out[b_i, th * T_CORE:(th + 1) * T_CORE, :] = \
            res[core]["out_t"].T.astype(np.float32)
    return out


# revision 3
# speedup vs baseline: 125.9313x; 1.2150x over previous
"""Trainium2 Bass kernel for CrossAttention (RoPE, 16 heads, D=1024) — v2.

Sharding: data-parallel over (batch, query-half): core c handles batch c//2,
query rows [1024*(c%2), 1024*(c%2+1)).  No cross-core communication; the
host gather is a pure concatenation.

v2 changes vs baseline:
  - all-bf16 data path (host pre-converts inputs/weights): halves DMA and
    SBUF, 2x DVE throughput, same 1 col/cycle matmul stream rate
  - q and A^T stay SBUF-resident (no q_t/a_t DRAM round trips)
  - PV matmul in fp8e4 + DoubleRow: contracts 256 kv rows per matmul
    (2 s-blocks interleaved), halving PV matmul count and cycles
  - all weights loaded up front; per-pair tile granularity so the Tile
    scheduler overlaps k/v-projection of quad q+1 under the ACT-bound
    attention of pair p
  - PSUM: psS bufs=2 x [128,1024] (4 banks) + psV bufs=4 x [65,512]
    (4 banks) = exactly 8 banks
"""

import sys
import numpy as np

sys.path.insert(0, "/opt/trn_rl_repo")

import concourse.bacc as bacc  # noqa: E402
import concourse.tile as tile  # noqa: E402
from concourse import mybir  # noqa: E402

F32 = mybir.dt.float32
BF16 = mybir.dt.bfloat16
FP8 = mybir.dt.float8e4
AF = mybir.ActivationFunctionType
PM = mybir.MatmulPerfMode

NHEAD = 16
DH = 64
B = 4
TQ = 2048
TKV = 2048
D = 1024
T_CORE = TQ // 2
N_CORES = 8

NJ = 8          # 128-row feature blocks
T = T_CORE      # 1024 query rows per core
S = TKV         # 2048 kv rows
NSB = S // 128  # 16 kv s-blocks
NSBP = NSB // 2  # 8 s-block pairs (DoubleRow)
NPAIR = 8       # head pairs
SCALE = 1.0 / float(np.sqrt(DH))


def emit(nc, tc, hd, phases=('q', 'kv', 'attn', 'out')):
    dmaq = [nc.sync, nc.gpsimd]

    with (
        tc.tile_pool(name="consts", bufs=1) as consts,
        tc.tile_pool(name="wpool", bufs=1) as wpool,
        tc.tile_pool(name="xin", bufs=1) as xin,
        tc.tile_pool(name="qres", bufs=1) as qres,
        tc.tile_pool(name="ares", bufs=1) as ares,
        tc.tile_pool(name="ktres", bufs=1) as ktres,
        tc.tile_pool(name="vqres", bufs=1) as vqres,
    ):
        # ---- input / weight DMAs (sync: activations, scalar: weights) ----
        x_t = []
        for i in range(NJ):
            t = xin.tile([128, T], BF16, tag=f"x{i}")
            nc.sync.dma_start(t[:], hd["xT"][i * 128:(i + 1) * 128, :])
            x_t.append(t)
        crepq = consts.tile([128, T], BF16, tag="crepq")
        nc.sync.dma_start(crepq[:], hd["crepq"][:])
        ssinq = consts.tile([128, T], BF16, tag="ssinq")
        nc.sync.dma_start(ssinq[:], hd["ssinq"][:])
        bq_sb = consts.tile([128, NJ], F32, tag="bq")
        nc.sync.dma_start(bq_sb[:], hd["bq_t"][:])

        wq_t = []
        for j in range(NJ):
            t = wpool.tile([128, NJ * 128], BF16, tag=f"wq{j}")
            nc.scalar.dma_start(
                t[:].rearrange("p (a c) -> p a c", a=NJ),
                hd["wq"][:, j * 128:(j + 1) * 128]
                .rearrange("(a p) c -> p a c", p=128),
            )
            wq_t.append(t)

        ctx_t = []
        for i in range(NJ):
            t = xin.tile([128, S], BF16, tag=f"ctx{i}")
            nc.sync.dma_start(t[:], hd["ctxT"][i * 128:(i + 1) * 128, :])
            ctx_t.append(t)
        crepk = consts.tile([128, S], BF16, tag="crepk")
        nc.sync.dma_start(crepk[:], hd["crepk"][:])
        ssink = consts.tile([128, S], BF16, tag="ssink")
        nc.sync.dma_start(ssink[:], hd["ssink"][:])
        bk_sb = consts.tile([128, NJ], F32, tag="bk")
        nc.sync.dma_start(bk_sb[:], hd["bk_t"][:])

        wk_t = []
        for j in range(NJ):
            t = wpool.tile([128, NJ * 128], BF16, tag=f"wk{j}")
            nc.scalar.dma_start(
                t[:].rearrange("p (a c) -> p a c", a=NJ),
                hd["wk"][:, j * 128:(j + 1) * 128]
                .rearrange("(a p) c -> p a c", p=128),
            )
            wk_t.append(t)
        wv_t = []
        for i in range(NJ):
            t = wpool.tile([128, D], BF16, tag=f"wv{i}")
            nc.scalar.dma_start(t[:], hd["wv"][i * 128:(i + 1) * 128, :])
            wv_t.append(t)
        bv_sb = consts.tile([128, D], F32, tag="bv")
        nc.scalar.dma_start(bv_sb[:], hd["bv_bcast"][:])
        ones_sb = consts.tile([128, 64], BF16, tag="ones")
        nc.scalar.dma_start(ones_sb[:], hd["ones_c"][:])
        wo_t = wpool.tile([128, NJ * D], BF16, tag="wo")
        nc.scalar.dma_start(
            wo_t[:].rearrange("p (a c) -> p a c", a=NJ),
            hd["wo"][:].rearrange("(a p) c -> p a c", p=128),
        )
        bo_sb = consts.tile([128, NJ], F32, tag="bo")
        nc.scalar.dma_start(bo_sb[:], hd["bo_t"][:])

        q_t = [qres.tile([128, T], BF16, tag=f"q{j}", name=f"q{j}")
               for j in range(NJ)]
        a_t = [ares.tile([128, T], BF16, tag=f"a{j}", name=f"a{j}")
               for j in range(NJ)]
        kt_t = [ktres.tile([128, S], BF16, tag=f"kt{j}", name=f"kt{j}")
                for j in range(NJ)]
        vq_t = [vqres.tile([128, NSBP * 2 * 4 * 80], FP8, tag=f"vq{qd}",
                           name=f"vq{qd}")
                for qd in range(4)]

        with (
            tc.tile_pool(name="psS", bufs=2, space="PSUM") as psS,
            tc.tile_pool(name="psV", bufs=4, space="PSUM") as psV,
            tc.tile_pool(name="rp", bufs=3) as rp,
            tc.tile_pool(name="esb", bufs=3) as esb,
            tc.tile_pool(name="zsb", bufs=4) as zsb,
            tc.tile_pool(name="ostg", bufs=4) as ostg,
        ):
            # ================= phase 1: q projection + rope ==============
            for j in range(NJ if 'q' in phases else 0):
                ps = psS.tile([128, T], F32, tag="ps")
                for c in range(2):
                    for i in range(NJ):
                        nc.tensor.matmul(
                            ps[:, c * 512:c * 512 + 512],
                            wq_t[j][:, i * 128:(i + 1) * 128],
                            x_t[i][:, c * 512:c * 512 + 512],
                            start=(i == 0), stop=(i == NJ - 1),
                        )
                raw = rp.tile([128, T], BF16, tag="raw")
                nc.vector.tensor_scalar_add(raw[:], ps[:], bq_sb[:, j:j + 1])
                shf = rp.tile([128, T], BF16, tag="shf")
                for g in range(4):
                    src = (g ^ 1) * 32
                    dmaq[g % 2].dma_start(shf[g * 32:g * 32 + 32, :],
                                          raw[src:src + 32, :])
                nc.vector.tensor_mul(raw[:], raw[:], crepq[:])
                nc.vector.tensor_mul(shf[:], shf[:], ssinq[:])
                nc.vector.tensor_add(q_t[j][:], raw[:], shf[:])

            # ========= per-quad kv projection + per-pair attention ========
            for qd in range(4 if 'kv' in phases else 0):
                # -- k projection for the quad's two pair-blocks --
                for jj in range(2):
                    j = qd * 2 + jj
                    for c in range(S // 512):
                        ps = psS.tile([128, 512], F32, tag="ps")
                        for i in range(NJ):
                            nc.tensor.matmul(
                                ps[:],
                                wk_t[j][:, i * 128:(i + 1) * 128],
                                ctx_t[i][:, c * 512:c * 512 + 512],
                                start=(i == 0), stop=(i == NJ - 1),
                            )
                        kraw = rp.tile([128, 512], BF16, tag="raw")
                        nc.vector.tensor_scalar_add(kraw[:], ps[:],
                                                    bk_sb[:, j:j + 1])
                        kshf = rp.tile([128, 512], BF16, tag="shf")
                        for g in range(4):
                            src = (g ^ 1) * 32
                            dmaq[g % 2].dma_start(kshf[g * 32:g * 32 + 32, :],
                                                  kraw[src:src + 32, :])
                        nc.vector.tensor_mul(
                            kraw[:], kraw[:], crepk[:, c * 512:c * 512 + 512])
                        nc.vector.tensor_mul(
                            kshf[:], kshf[:], ssink[:, c * 512:c * 512 + 512])
                        nc.vector.tensor_add(
                            kt_t[j][:, c * 512:c * 512 + 512], kraw[:], kshf[:])

                # -- v projection: s-major fp8, DR-interleaved, ones col --
                vql = vq_t[qd][:].rearrange("p (s t h d) -> p s t h d",
                                            s=NSBP, t=2, h=4)
                for sb in range(NSB):
                    sbp, par = sb // 2, sb % 2
                    ps = psS.tile([128, 256], F32, tag="ps")
                    for i in range(NJ):
                        nc.tensor.matmul(
                            ps[:],
                            ctx_t[i][:, sb * 128:sb * 128 + 128],
                            wv_t[i][:, qd * 256:(qd + 1) * 256],
                            start=(i == 0), stop=(i == NJ - 1),
                        )
                    nc.vector.tensor_add(
                        vql[:, sbp, par, :, 0:64],
                        ps[:].rearrange("p (h d) -> p h d", d=64),
                        bv_sb[:, qd * 256:(qd + 1) * 256]
                        .rearrange("p (h d) -> p h d", d=64),
                    )
                nc.vector.tensor_copy(
                    vql[:, :, :, :, 64:65],
                    ones_sb[:].rearrange("p (s t h) -> p s t h", s=NSBP, t=2)
                    [:, :, :, :, None],
                )

                # -- attention for the quad's two head pairs --
                for jj in range(2 if 'attn' in phases else 0):
                    pair = qd * 2 + jj
                    pv_ps = [[psV.tile([65, 512], F32, tag="pv", name="pv")
                              for _ in range(2)] for _ in range(2)]
                    for sbp in range(NSBP):
                        e2 = [None, None]
                        for par in range(2):
                            e2[par] = esb.tile([128, 2 * T], FP8, tag="e2", name="e2")
                        for parity in range(2):
                            sb = sbp * 2 + parity
                            for par in range(2):
                                rows = slice(par * 64, par * 64 + 64)
                                sps = psS.tile([128, T], F32, tag="ps")
                                for c in range(2):
                                    nc.tensor.matmul(
                                        sps[:, c * 512:c * 512 + 512],
                                        kt_t[pair][rows,
                                                   sb * 128:sb * 128 + 128],
                                        q_t[pair][rows,
                                                  c * 512:c * 512 + 512],
                                        start=True, stop=True,
                                    )
                                nc.scalar.activation(
                                    e2[par][:, parity * T:parity * T + T],
                                    sps[:], AF.Exp, scale=SCALE)
                        for par in range(2):
                            hq = jj * 2 + par
                            lhs = vql[:, sbp, :, hq, 0:65]
                            for c in range(2):
                                nc.tensor.matmul(
                                    pv_ps[par][c][:],
                                    lhs,
                                    e2[par][:].rearrange(
                                        "p (t n) -> p t n", t=2)
                                    [:, :, c * 512:c * 512 + 512],
                                    start=(sbp == 0), stop=(sbp == NSBP - 1),
                                    perf_mode=PM.DoubleRow,
                                )
                    for par in range(2):
                        for c in range(2):
                            ps = pv_ps[par][c]
                            zinv = zsb.tile([1, 512], F32, tag="zinv")
                            nc.vector.reciprocal(zinv[:], ps[64:65, :])
                            bc = zsb.tile([64, 512], F32, tag="bc")
                            nc.gpsimd.partition_broadcast(bc[:], zinv[:])
                            nc.vector.tensor_mul(
                                a_t[pair][par * 64:par * 64 + 64,
                                          c * 512:c * 512 + 512],
                                ps[0:64, :], bc[:])

            # ================= phase 4: output projection =================
            for e in range(NJ if 'out' in phases else 0):
                for c in range(2):
                    ps = psS.tile([128, 512], F32, tag="ps")
                    for i in range(NJ):
                        nc.tensor.matmul(
                            ps[:],
                            wo_t[:, i * D + e * 128:i * D + e * 128 + 128],
                            a_t[i][:, c * 512:c * 512 + 512],
                            start=(i == 0), stop=(i == NJ - 1),
                        )
                    ot = ostg.tile([128, 512], BF16, tag="ot")
                    nc.vector.tensor_scalar_add(ot[:], ps[:], bo_sb[:, e:e + 1])
                    dmaq[(e * 2 + c) % 2].dma_start(
                        hd["out_t"][e * 128:(e + 1) * 128,
                                    c * 512:c * 512 + 512],
                        ot[:],
                    )


def build(reps=1, phases=('q', 'kv', 'attn', 'out')):
    nc = bacc.Bacc("TRN2", target_bir_lowering=False, debug=False)
    hd = {}
    for name, shape, dt in [
        ("xT", [D, T], BF16), ("ctxT", [D, S], BF16),
        ("wq", [D, D], BF16), ("wk", [D, D], BF16),
        ("wv", [D, D], BF16), ("wo", [D, D], BF16),
        ("crepq", [128, T], BF16), ("ssinq", [128, T], BF16),
        ("crepk", [128, S], BF16), ("ssink", [128, S], BF16),
        ("bq_t", [128, NJ], F32), ("bk_t", [128, NJ], F32),
        ("bv_bcast", [128, D], F32), ("bo_t", [128, NJ], F32),
        ("ones_c", [128, 64], BF16),
    ]:
        hd[name] = nc.dram_tensor(name, shape, dt, kind="ExternalInput")
    hd["out_t"] = nc.dram_tensor("out_t", [D, T], BF16, kind="ExternalOutput")

    with tile.TileContext(nc) as tc:
        for _ in range(reps):
            emit(nc, tc, hd, phases=phases)
    nc.compile()
    return nc


def host_prep(x, context, Wq, bq, Wkv, bkv, Wo, bo, cos_tab, sin_tab):
    """Build the per-core input maps (layout + dtype conversion only)."""
    import ml_dtypes
    BF = ml_dtypes.bfloat16
    Dm = D
    perm = np.concatenate(
        [h * DH + np.concatenate([np.arange(0, DH, 2), np.arange(1, DH, 2)])
         for h in range(NHEAD)])
    c = np.ascontiguousarray
    wq = c(Wq[perm, :].T.astype(BF))
    wk = c(Wkv[0:Dm][perm, :].T.astype(BF))
    wv = c(Wkv[Dm:2 * Dm].T.astype(BF))
    wo = c(Wo.T.astype(BF))
    bq_t = c(bq[perm].reshape(NJ, 128).T.astype(np.float32))
    bk_t = c(bkv[0:Dm][perm].reshape(NJ, 128).T.astype(np.float32))
    bv_bcast = c(np.tile(bkv[Dm:2 * Dm].reshape(1, Dm), (128, 1))
                 .astype(np.float32))
    bo_t = c(bo.reshape(NJ, 128).T.astype(np.float32))

    def mk_tables(lo, hi):
        ct = cos_tab[lo:hi].T.astype(np.float32)
        st = sin_tab[lo:hi].T.astype(np.float32)
        return (c(np.tile(ct, (4, 1)).astype(BF)),
                c(np.concatenate([-st, st, -st, st], axis=0).astype(BF)))

    crepk, ssink = mk_tables(0, S)
    shared = dict(wq=wq, wk=wk, wv=wv, wo=wo, bq_t=bq_t, bk_t=bk_t,
                  bv_bcast=bv_bcast, bo_t=bo_t, crepk=crepk, ssink=ssink,
                  ones_c=np.ones((128, 64), BF))
    in_maps = []
    for core in range(N_CORES):
        b_i, th = divmod(core, 2)
        crepq, ssinq = mk_tables(th * T, (th + 1) * T)
        m = dict(shared)
        m.update(
            xT=c(x[b_i, th * T:(th + 1) * T, :].T.astype(BF)),
            ctxT=c(context[b_i].T.astype(BF)),
            crepq=crepq, ssinq=ssinq,
        )
        in_maps.append(m)
    return in_maps


_NC_CACHE = {}


def get_nc():
    if "nc" not in _NC_CACHE:
        _NC_CACHE["nc"] = build()
    return _NC_CACHE["nc"]


def make_runner(nc, n_cores=N_CORES):
    """Build a reusable jitted SPMD executor (device-resident inputs)."""
    import jax
    from jax.experimental.shard_map import shard_map
    from jax.sharding import Mesh, NamedSharding, PartitionSpec
    from concourse import bass2jax, mybir as _mybir

    bass2jax.install_neuronx_cc_hook()
    part_name = (nc.partition_id_tensor.name
                 if nc.partition_id_tensor else None)
    in_names, out_names, out_avals = [], [], []
    for alloc in nc.m.functions[0].allocations:
        if not isinstance(alloc, _mybir.MemoryLocationSet):
            continue
        name = alloc.memorylocations[0].name
        if alloc.kind == "ExternalInput":
            if name == part_name:
                continue
            in_names.append(name)
        elif alloc.kind == "ExternalOutput":
            out_names.append(name)
            out_avals.append(jax.core.ShapedArray(
                tuple(alloc.tensor_shape), _mybir.dt.np(alloc.dtype)))
    n_params = len(in_names)
    all_in = in_names + out_names
    if part_name is not None:
        all_in = all_in + [part_name]

    def _body(*args):
        ops = list(args)
        if part_name is not None:
            ops.append(bass2jax.partition_id_tensor())
        outs = bass2jax._bass_exec_p.bind(
            *ops,
            out_avals=tuple(out_avals),
            in_names=tuple(all_in),
            out_names=tuple(out_names),
            lowering_input_output_aliases=(),
            sim_require_finite=True,
            sim_require_nnan=True,
            nc=nc,
        )
        return tuple(outs)

    devices = jax.devices()[:n_cores]
    mesh = Mesh(np.asarray(devices), ("core",))
    nouts = len(out_names)
    sharded = jax.jit(
        shard_map(_body, mesh=mesh,
                  in_specs=(PartitionSpec("core"),) * (n_params + nouts),
                  out_specs=(PartitionSpec("core"),) * nouts,
                  check_rep=False),
        keep_unused=True,
    )
    sh = NamedSharding(mesh, PartitionSpec("core"))

    def put(in_maps):
        args = [np.concatenate([m[name] for m in in_maps], axis=0)
                for name in in_names[:n_params]]
        for av in out_avals:
            args.append(np.zeros((n_cores * av.shape[0],) + av.shape[1:],
                                 av.dtype))
        return [jax.device_put(a, sh) for a in args]

    def run(args):
        outs = sharded(*args)
        jax.block_until_ready(outs)
        return outs

    def gather(outs):
        return [
            {name: np.asarray(outs[i]).reshape(n_cores, *out_avals[i].shape)[c]
             for i, name in enumerate(out_names)}
            for c in range(n_cores)
        ]

    return put, run, gather


def get_runner():
    if "runner" not in _NC_CACHE:
        _NC_CACHE["runner"] = make_runner(get_nc())
    return _NC_CACHE["runner"]


def kernel(x, context, Wq, bq, Wkv, bkv, Wo, bo, cos_tab, sin_tab):
    args = [np.asarray(a, dtype=np.float32) for a in
            (x, context, Wq, bq, Wkv, bkv, Wo, bo, cos_tab, sin_tab)]
    in_maps = host_prep(*args)
    put, run, gather = get_runner()
    res = gather(run(put(in_maps)))
    out = np.empty((B, TQ, D), dtype=np.float32)
    for core in range(N_CORES):
        b_i, th = divmod(core, 2)
        out[b_i, th * T_CORE:(th + 1) * T_CORE, :] = \
            res[core]["out_t"].T.astype(np.float32)
    return out
